# revision 33
# baseline (speedup 1.0000x reference)
"""Trainium2 Bass kernel for BilinearClassification (segment_reduce).

Math (per example b):
  ent[e,:]  = masked-mean over subword span of hidden[idx[e,s],:]      (E=64, H=768)
  subj[t,:] = ent[trip[t,0],:] * pm[t];  obj[t,:] = ent[trip[t,1],:] * pm[t]
  bl[t, (g,i,j)] = subj[t, g*8+i] * obj[t, g*8+j]                      (f = 6144)
  logits[t,n] = bl[t,:] @ W[:,n] + b[n]                                (NT=42)

Device strategy (8 cores, 4 examples each, no collectives, all-bf16 inputs):
  - host precomputes one-hot segment-mean matrix AT[l, (ex,e)] (mask, 1/cnt and
    example-pair block-diagonal folded) and pair-stacked triplet one-hots
    G_s/G_o [128=(2ex x 64e), 256=(2ex x 128t)] (pair_mask folded), so the
    device only does matmuls + copies + one elementwise mul.
  - stage 1: ent = AT.T @ hidden; two 384-col PSUM halves per example pair.
  - tables: ent_exp_s[(ex,e), (g,i,j)] = ent[(ex,e), 8g+i] (and _o with 8g+j),
    built by broadcast-AP copies (ACT/DVE for the first slices, idle GPSIMD
    for the rest) because matmul weights APs must be 2D.
  - stage 2 per f-chunk c (128 f-rows = 2 bilinear groups x 8i x 8j):
      S_exp = ent_exp_s[:, c-slice].T @ G_s   -> PSUM [128, 256]
      O_exp = ent_exp_o[:, c-slice].T @ G_o   -> PSUM [128, 256]
      s_sb  = ACT evac of S_exp; blT_c = s_sb * O_exp (DVE) -> SBUF bf16
      logits += W_c.T @ blT_c                 (PSUM accumulate over 48 chunks)
  - host adds b_fc and reshapes.
"""
import sys

sys.path.insert(0, "/opt/trn_rl_repo")

import numpy as np

import concourse.bass as bass
import concourse.bacc as bacc
import concourse.tile as tile
from concourse import mybir
from concourse.bass_utils import run_bass_kernel_spmd

F32 = mybir.dt.float32
BF16 = mybir.dt.bfloat16

B, L, H = 32, 512, 768
E, S, T = 64, 8, 128
NT = 42
NCORES = 8
EXPC = B // NCORES          # 4 examples per core
NPAIR = EXPC // 2           # 2 example-pairs per core
KC = L // 128               # 4 contraction chunks over l
FC = (H * 8) // 128         # 48 f-chunks
TP = 2 * T                  # 256 columns per pair (2ex x 128t)
NH = 2                      # ent psum halves (384 cols each)
HHALF = H // NH


def build_program(reps=1):
    """reps>1 repeats the whole body back-to-back (for wall-clock timing
    amplification in the test harness; the grading path uses reps=1)."""
    nc = bacc.Bacc("TRN2", target_bir_lowering=False, debug=False)

    hid_d = nc.dram_tensor("hid", (EXPC * L, H), BF16, kind="ExternalInput")
    # AT is pair-block-diagonal: rows (ex,kc,l), cols (ex' * 64 + e)
    at_d = nc.dram_tensor("at", (EXPC * L, 2 * E), BF16, kind="ExternalInput")
    gs_d = nc.dram_tensor("gs", (NPAIR, 128, TP), BF16, kind="ExternalInput")
    go_d = nc.dram_tensor("go", (NPAIR, 128, TP), BF16, kind="ExternalInput")
    # W pre-shuffled on host to the SBUF chunk layout [p, (c n)]
    w_d = nc.dram_tensor("w", (128, FC * NT), BF16, kind="ExternalInput")
    out_d = nc.dram_tensor("out", (NPAIR, NT, TP), F32, kind="ExternalOutput")

    with tile.TileContext(nc) as tc:
        with (
            tc.tile_pool(name="consts", bufs=1) as consts,
            tc.tile_pool(name="hidp", bufs=2) as hidp,
            tc.tile_pool(name="atp", bufs=2) as atp,
            tc.tile_pool(name="entps", bufs=2, space="PSUM") as entps,
            tc.tile_pool(name="entsb", bufs=2) as entsb,
            tc.tile_pool(name="tabp", bufs=2) as tabp,
            tc.tile_pool(name="sops", bufs=2, space="PSUM") as sops,
            tc.tile_pool(name="lgps", bufs=2, space="PSUM") as lgps,
            tc.tile_pool(name="blp", bufs=6) as blp,
            tc.tile_pool(name="outp", bufs=2) as outp,
        ):
          for _rep in range(reps):
            # ---- input DMAs, consolidated (HWDGE setup is ~600ns per DMA)
            # and ordered so pair 0's ent inputs land first
            hid_t = [None] * NPAIR   # [128, 8ck, H] per pair
            at_t = [None] * NPAIR    # [128, 8ck, 2E] per pair
            gs_t = [None] * NPAIR
            go_t = [None] * NPAIR
            w_all = None
            for P in range(NPAIR):
                att = atp.tile([128, 2 * KC, 2 * E], BF16)
                at_r = at_d[:].rearrange("(ck p) e -> p ck e", p=128)
                nc.sync.dma_start(att[:], at_r[:, P * 2 * KC : (P + 1) * 2 * KC, :])
                at_t[P] = att
                hid_r = hid_d[:].rearrange("(ck p) h -> p ck h", p=128)
                hts = []
                for half in range(2):  # separate tiles so deps are per-half
                    h1 = hidp.tile([128, KC, H], BF16)
                    if P == 0 and half == 0:
                        # finest granularity on the critical first chunks so
                        # the very first ent matmul starts as early as possible
                        for ck in range(KC):
                            nc.sync.dma_start(
                                h1[:, ck, :], hid_r[:, P * 2 * KC + ck, :])
                    else:
                        nc.sync.dma_start(
                            h1[:],
                            hid_r[:, P * 2 * KC + half * KC : P * 2 * KC + (half + 1) * KC, :])
                    hts.append(h1)
                hid_t[P] = hts
                g1 = consts.tile([128, TP], BF16, tag=f"gs{P}")
                nc.sync.dma_start(g1[:], gs_d[P])
                gs_t[P] = g1
                g2 = consts.tile([128, TP], BF16, tag=f"go{P}")
                nc.sync.dma_start(g2[:], go_d[P])
                go_t[P] = g2
                if P == 0:
                    w_all = consts.tile([128, FC, NT], BF16, tag="w")
                    nc.sync.dma_start(
                        w_all[:], w_d[:].rearrange("p (c n) -> p c n", n=NT))

            # ---- stage 1 + tables for every pair first (program order)
            tabs = []
            for P in range(NPAIR):
                ent_sb = entsb.tile([128, H], BF16, tag="ent_sb")
                tab_s = tabp.tile([128, H * 8], BF16, tag="tab_s")
                tab_o = tabp.tile([128, H * 8], BF16, tag="tab_o")
                for nh in range(NH):
                    fast = P == 0 and nh == 0
                    ent_ps = entps.tile([128, HHALF], F32)
                    for ck in range(2 * KC):
                        nc.tensor.matmul(
                            ent_ps[:],
                            at_t[P][:, ck, :],
                            hid_t[P][ck // KC][:, ck % KC,
                                               nh * HHALF : (nh + 1) * HHALF],
                            start=(ck == 0),
                            stop=(ck == 2 * KC - 1),
                        )
                    if not fast:
                        # GPSIMD has no PSUM port: stage this half into SBUF
                        nc.scalar.copy(
                            ent_sb[:, nh * HHALF : (nh + 1) * HHALF], ent_ps[:])
                    # table slices covering this half: groups [nh*48, (nh+1)*48)
                    g0 = nh * (96 // NH)
                    gn2 = 96 // NH // 2   # two slices per half
                    for sl in range(2):
                        ga = g0 + sl * gn2
                        if fast:
                            # straight from PSUM on the idle fast engines so
                            # pair 0 stage 2 starts as early as possible
                            src = ent_ps[:, (ga - g0) * 8 : (ga - g0 + gn2) * 8]
                        else:
                            src = ent_sb[:, ga * 8 : (ga + gn2) * 8]
                        src_s = (src.rearrange("p (g i) -> p g i", i=8)
                                 .unsqueeze(3).broadcast_to((128, gn2, 8, 8)))
                        src_o = (src.rearrange("p (g j) -> p g j", j=8)
                                 .unsqueeze(2).broadcast_to((128, gn2, 8, 8)))
                        dst_s = tab_s[:, ga * 64 : (ga + gn2) * 64].rearrange(
                            "p (g i j) -> p g i j", i=8, j=8)
                        dst_o = tab_o[:, ga * 64 : (ga + gn2) * 64].rearrange(
                            "p (g i j) -> p g i j", i=8, j=8)
                        if fast:
                            nc.scalar.copy(dst_s, src_s)
                            nc.vector.tensor_copy(dst_o, src_o)
                        else:
                            # on GPSIMD, overlapped with running stage 2
                            nc.gpsimd.tensor_copy(dst_s, src_s)
                            nc.gpsimd.tensor_copy(dst_o, src_o)
                tabs.append((tab_s, tab_o))

            # ---- stage 2: 48 f-chunks per pair, merged in pairs of chunks
            for P in range(NPAIR):
                tab_s, tab_o = tabs[P]
                lg_ps = lgps.tile([NT, TP], F32)
                for cc in range(FC // 2):
                    s_ps = sops.tile([128, 2, TP], F32, tag="s")
                    o_ps = sops.tile([128, 2, TP], F32, tag="o")
                    for h in range(2):
                        c = cc * 2 + h
                        nc.tensor.matmul(
                            s_ps[:, h, :],
                            tab_s[:, c * 128 : (c + 1) * 128],
                            gs_t[P][:],
                            start=True,
                            stop=True,
                        )
                        nc.tensor.matmul(
                            o_ps[:, h, :],
                            tab_o[:, c * 128 : (c + 1) * 128],
                            go_t[P][:],
                            start=True,
                            stop=True,
                        )
                    # DVE can read at most one PSUM operand: evacuate S via ACT
                    s_sb = blp.tile([128, 2, TP], F32, tag="s_sb")
                    nc.scalar.copy(s_sb[:], s_ps[:])
                    blt = blp.tile([128, 2, TP], BF16, tag="blt")
                    nc.vector.tensor_mul(blt[:], s_sb[:], o_ps[:])
                    for h in range(2):
                        c = cc * 2 + h
                        nc.tensor.matmul(
                            lg_ps[:],
                            w_all[:, c, :],
                            blt[:, h, :],
                            start=(c == 0),
                            stop=(c == FC - 1),
                        )

                out_sb = outp.tile([NT, TP], F32)
                nc.scalar.copy(out_sb[:], lg_ps[:])
                nc.sync.dma_start(out_d[P], out_sb[:])

    nc.compile()
    return nc


def host_prep(hidden_states, entity_subw_indices, entity_subw_mask,
              triplet_entity_nums, pair_mask, W_fc):
    """Build per-core input maps (numpy only, cheap)."""
    import ml_dtypes
    bf16 = ml_dtypes.bfloat16
    hs = np.asarray(hidden_states, dtype=np.float32).astype(bf16)
    idx = np.asarray(entity_subw_indices)
    msk = np.asarray(entity_subw_mask).astype(np.float32)
    trip = np.asarray(triplet_entity_nums)
    pm = np.asarray(pair_mask).astype(np.float32)
    # shuffle W to the SBUF chunk layout [p, (c, n)]
    w = (np.asarray(W_fc, dtype=np.float32).reshape(FC, 128, NT)
         .transpose(1, 0, 2).reshape(128, FC * NT).astype(bf16))

    # AT[b]: (L, 2E) pair-block-diagonal with mask/cnt folded
    cnt = np.maximum(msk.sum(axis=2), 1.0)          # (B, E)
    wgt = msk / cnt[:, :, None]                     # (B, E, S)
    at = np.zeros((B, L, 2 * E), np.float32)
    b_i, e_i, s_i = np.nonzero(msk > 0)
    np.add.at(at, (b_i, idx[b_i, e_i, s_i], (b_i % 2) * E + e_i),
              wgt[b_i, e_i, s_i])
    at = at.astype(bf16)

    # pair-stacked block-diagonal triplet one-hots (2ex x 64e, 2ex x 128t)
    gs = np.zeros((B // 2, 128, TP), bf16)
    go = np.zeros((B // 2, 128, TP), bf16)
    bb = np.arange(B)[:, None]
    tt = np.arange(T)[None, :]
    pair = bb // 2
    exl = (bb % 2)
    gs[pair, exl * E + trip[:, :, 0], exl * T + tt] = pm.astype(bf16)
    go[pair, exl * E + trip[:, :, 1], exl * T + tt] = pm.astype(bf16)

    in_maps = []
    for c in range(NCORES):
        b0 = c * EXPC
        in_maps.append({
            "hid": np.ascontiguousarray(hs[b0 : b0 + EXPC].reshape(EXPC * L, H)),
            "at": np.ascontiguousarray(
                at[b0 : b0 + EXPC].reshape(EXPC * L, 2 * E)),
            "gs": np.ascontiguousarray(gs[b0 // 2 : b0 // 2 + NPAIR]),
            "go": np.ascontiguousarray(go[b0 // 2 : b0 // 2 + NPAIR]),
            "w": w,
        })
    return in_maps


def assemble(results, b_fc):
    """results[c]["out"] is (NPAIR, NT, 2ex x 128t) -> (B, T, NT) + bias."""
    logits = np.empty((B, T, NT), np.float32)
    for c in range(NCORES):
        o = results[c]["out"].reshape(NPAIR, NT, 2, T)
        for P in range(NPAIR):
            for exl in range(2):
                b = c * EXPC + P * 2 + exl
                logits[b] = o[P, :, exl, :].T
    return logits + np.asarray(b_fc, np.float32)[None, None, :]


_NC_CACHE = None


def kernel(hidden_states, entity_subw_indices, entity_subw_mask,
           triplet_entity_nums, pair_mask, W_fc, b_fc):
    global _NC_CACHE
    if _NC_CACHE is None:
        _NC_CACHE = build_program()
    nc = _NC_CACHE
    in_maps = host_prep(hidden_states, entity_subw_indices, entity_subw_mask,
                        triplet_entity_nums, pair_mask, W_fc)
    res = run_bass_kernel_spmd(nc, in_maps, core_ids=list(range(NCORES)))
    return assemble(res.results, b_fc)
